# revision 13
# baseline (speedup 1.0000x reference)
"""Trainium2 Bass kernel for the pairwise-cosine masked ratio loss.

reference semantics:
    g  = min-max-normalized grad rows          [B, D]
    cos_g, cos_x = pairwise cosine Gram matrices
    loss = sum over same-class pairs i<j of (1-cos_g)/(1-cos_x) / B

Key facts used:
  * cosine is invariant to positive per-row affine scale, so min-max
    normalization reduces to u = (g - rowmin(g)) / ||g - rowmin(g)||.
  * the mask (same argmax class, i<j) makes the Gram sum block-diagonal
    after sorting rows by class; ratio matrix is symmetric, so
    loss = (sum over all same-class ordered pairs i != j) / 2 / B.

Sharding: rows sorted by class (class order chosen to minimize the
max per-core class-span), 512 contiguous sorted rows per core.  Each
core receives its 512 rows plus the rest of the class spans they touch
("column block", rotated so the core's own rows come first), and a
host-built mask [512, NCOL] encoding same-class & not-diagonal &
real-column.  Every same-class ordered pair (i,j) is produced by
exactly one core (the owner of row i), so the masked sum over all
cores counts each unordered pair exactly twice.

Device program (SPMD, identical program for all cores; data differs):
  phase 1 (per matrix): per 128-row tile: row min (g only, fused
           negate), Square activation with accum -> sum of squares;
           batched sqrt + one reciprocal; u = (v - min) * inv -> bf16;
           PE-transpose into U^T [128, KT, NCOL].
  phase 2: per 128-row m-tile: Gram blocks for g and x via bf16
           matmuls (K=1024 in 8 k-tiles, PSUM f32),
           sx = min(cos_x - 1, -1e-30)  (clamp protects masked pairs),
           num = (cos_g - 1) * mask     (DVE),
           partial += sum(num / sx)     (fast-approx reciprocal + fused
           multiply-accumulate on DVE) == sum mask*(1-cos_g)/(1-cos_x).
  finale:  partition-reduce partials via matmul with ones -> [1,1].
Host sums the 8 partial scalars, divides by 2*B.
"""

import numpy as np

import concourse.bass as bass
import concourse.bacc as bacc
import concourse.mybir as mybir
import concourse.tile as tile
from concourse import bass_utils

B = 4096
D = 1024
NCORES = 8
NR = B // NCORES          # 512 own rows per core
KT = D // 128             # k-tiles
MT = NR // 128            # m-tiles per core
F32 = mybir.dt.float32
BF16 = mybir.dt.bfloat16
AF = mybir.ActivationFunctionType
ALU = mybir.AluOpType
AX = mybir.AxisListType


def _build_program(ncol: int) -> bacc.Bacc:
    nc = bacc.Bacc("TRN2", target_bir_lowering=False, debug=False,
                   num_devices=NCORES)
    gcols = nc.dram_tensor("gcols", [ncol, D], BF16, kind="ExternalInput")
    xcols = nc.dram_tensor("xcols", [ncol, D], BF16, kind="ExternalInput")
    maskd = nc.dram_tensor("mask", [NR, ncol], BF16, kind="ExternalInput")
    ident = nc.dram_tensor("ident", [128, 128], BF16, kind="ExternalInput")
    outd = nc.dram_tensor("out", [1, 1], F32, kind="ExternalOutput")

    T = ncol // 128
    TOWN = NR // 128        # tiles holding the core's own rows
    # triangle scheme: m-tile mi only computes columns >= mi*128 (the
    # skipped region is the own-block lower triangle; the host mask
    # counts own-block upper pairs with weight 2.0 instead).
    # own segments [mi*128, NR) depend only on tiles 0..TOWN-1, so they
    # are emitted before the boundary tiles are even normalized --
    # software-pipelining phase 2 into phase 1.
    own_segs = [(mi, mi * 128, NR - mi * 128) for mi in range(MT)]
    bnd_segs = []
    for mi in range(MT):
        cs = NR
        while cs < ncol:
            cw = min(512, ncol - cs)
            bnd_segs.append((mi, cs, cw))
            cs += cw
    npart = len(own_segs) + len(bnd_segs)

    with tile.TileContext(nc) as tc:
        with (
            tc.tile_pool(name="cst", bufs=1) as cst,
            tc.tile_pool(name="io", bufs=6) as io,
            tc.tile_pool(name="ut", bufs=1) as utp,
            tc.tile_pool(name="sm", bufs=2) as smp,
            tc.tile_pool(name="wk", bufs=3) as wk,
            tc.tile_pool(name="tp", bufs=2, space="PSUM") as psp,
            tc.tile_pool(name="gr", bufs=2, space="PSUM") as psg,
            tc.tile_pool(name="fi", bufs=1, space="PSUM") as psf,
        ):
            identt = cst.tile([128, 128], BF16, name="identt")
            nc.sync.dma_start(identt[:], ident[:])
            parts = cst.tile([128, npart], F32, name="parts")
            utg = utp.tile([128, KT, ncol], BF16, name="utg")
            utx = utp.tile([128, KT, ncol], BF16, name="utx")
            # per-row stats; ssq/inv column index = 2*t + m (m: 0=G 1=X)
            nm = cst.tile([128, T], F32, name="nm")
            ssq = cst.tile([128, 2 * T], F32, name="ssq")
            inv = cst.tile([128, 2 * T], F32, name="inv")

            def phase1_group(grp):
                """Normalize + transpose tiles in `grp` for G and X with
                one batched sqrt+reciprocal for the group."""
                raws = {}
                for t in grp:
                    for m, src in ((0, gcols), (1, xcols)):
                        raw = io.tile([128, D], BF16, tag="raw",
                                      name="raw")
                        nc.sync.dma_start(raw[:],
                                          src[t * 128:(t + 1) * 128, :])
                        raws[(m, t)] = raw
                        sq = wk.tile([128, D], BF16, tag="sq", name="sq")
                        c = 2 * t + m
                        if m == 0:
                            # two-stage row min (TT halves then reduce)
                            mh = wk.tile([128, D // 2], BF16, tag="mh",
                                         name="mh")
                            nc.vector.tensor_tensor(
                                mh[:], raw[:, :D // 2], raw[:, D // 2:],
                                op=ALU.min)
                            nc.vector.tensor_reduce(nm[:, t:t + 1], mh[:],
                                                    axis=AX.X, op=ALU.min,
                                                    negate=True)
                            nc.scalar.activation(sq[:], raw[:], AF.Square,
                                                 bias=nm[:, t:t + 1],
                                                 scale=1.0,
                                                 accum_out=ssq[:, c:c + 1])
                        else:
                            nc.scalar.activation(sq[:], raw[:], AF.Square,
                                                 bias=0.0, scale=1.0,
                                                 accum_out=ssq[:, c:c + 1])
                c0, cn = 2 * grp[0], 2 * len(grp)
                nrm = smp.tile([128, 2 * T], F32, tag="nrm", name="nrm")
                nc.scalar.sqrt(nrm[:, c0:c0 + cn], ssq[:, c0:c0 + cn])
                nc.vector.reciprocal(inv[:, c0:c0 + cn],
                                     nrm[:, c0:c0 + cn])
                for t in grp:
                    for m, ut in ((0, utg), (1, utx)):
                        c = 2 * t + m
                        u = wk.tile([128, D], BF16, tag="u", name="u")
                        if m == 0:
                            nc.vector.tensor_scalar(u[:], raws[(m, t)][:],
                                                    nm[:, t:t + 1],
                                                    inv[:, c:c + 1],
                                                    op0=ALU.add,
                                                    op1=ALU.mult)
                        else:
                            nc.vector.tensor_scalar_mul(u[:],
                                                        raws[(m, t)][:],
                                                        inv[:, c:c + 1])
                        ps = psp.tile([128, D], BF16, tag="tp", name="ps")
                        for kk in range(KT):
                            nc.tensor.transpose(
                                ps[:, kk * 128:(kk + 1) * 128],
                                u[:, kk * 128:(kk + 1) * 128],
                                identt[:])
                        cp = nc.scalar.copy if t % 2 == 0 else \
                            nc.vector.tensor_copy
                        cp(
                            ut[:, :, t * 128:(t + 1) * 128],
                            ps[:].rearrange("p (k c) -> p k c", k=KT),
                        )

            def emit_seg(mi, cs, cw, pidx):
                maskt = wk.tile([128, 512], BF16, tag="maskt",
                                name="maskt")
                nc.sync.dma_start(
                    maskt[:, :cw],
                    maskd[mi * 128:(mi + 1) * 128, cs:cs + cw])
                pg = psg.tile([128, 512], F32, tag="pg", name="pg")
                px = psg.tile([128, 512], F32, tag="px", name="px")
                for kk in range(KT):
                    nc.tensor.matmul(
                        pg[:, :cw],
                        utg[:, kk, mi * 128:(mi + 1) * 128],
                        utg[:, kk, cs:cs + cw],
                        start=(kk == 0), stop=(kk == KT - 1))
                for kk in range(KT):
                    nc.tensor.matmul(
                        px[:, :cw],
                        utx[:, kk, mi * 128:(mi + 1) * 128],
                        utx[:, kk, cs:cs + cw],
                        start=(kk == 0), stop=(kk == KT - 1))
                sx = wk.tile([128, 512], F32, tag="sx", name="sx")
                nc.vector.tensor_scalar(sx[:, :cw], px[:, :cw], 1.0,
                                        -1e-30, op0=ALU.subtract,
                                        op1=ALU.min)
                rx = wk.tile([128, 512], F32, tag="rx", name="rx")
                nc.vector.reciprocal_approx_fast(rx[:, :cw], sx[:, :cw])
                num = wk.tile([128, 512], F32, tag="num", name="num")
                nc.vector.scalar_tensor_tensor(
                    num[:, :cw], pg[:, :cw], 1.0, maskt[:, :cw],
                    op0=ALU.subtract, op1=ALU.mult)
                junk = wk.tile([128, 512], F32, tag="junk", name="junk")
                # (num * 1) * rx, accum_out = sum -> partial
                nc.vector.scalar_tensor_tensor(
                    junk[:, :cw], num[:, :cw], 1.0, rx[:, :cw],
                    op0=ALU.mult, op1=ALU.mult,
                    accum_out=parts[:, pidx:pidx + 1])

            # ---- pipelined emission ----
            # own tiles, then own-column Gram segments interleaved with
            # the boundary tiles, then boundary segments
            for t0 in range(0, TOWN, 2):
                phase1_group(list(range(t0, min(t0 + 2, TOWN))))
            bnd_tiles = list(range(TOWN, T))
            for i, (mi, cs, cw) in enumerate(own_segs):
                emit_seg(mi, cs, cw, i)
                if i * 2 < len(bnd_tiles):
                    phase1_group(bnd_tiles[i * 2:i * 2 + 2])
            for i in range(len(own_segs) * 2, len(bnd_tiles), 2):
                phase1_group(bnd_tiles[i:i + 2])
            for i, (mi, cs, cw) in enumerate(bnd_segs):
                emit_seg(mi, cs, cw, len(own_segs) + i)

            # ---- finale: reduce partials to one scalar ----
            total = smp.tile([128, 1], F32, tag="total", name="total")
            nc.vector.reduce_sum(total[:], parts[:], axis=AX.X)
            ones = cst.tile([128, 1], F32, name="ones")
            nc.vector.memset(ones[:], 1.0)
            fin = psf.tile([1, 1], F32, name="fin")
            nc.tensor.matmul(fin[:], total[:], ones[:])
            outs = smp.tile([1, 1], F32, tag="outs", name="outs")
            nc.scalar.copy(outs[:], fin[:])
            nc.sync.dma_start(outd[:], outs[:])

    nc.compile()
    return nc


_PROGRAM_CACHE: dict = {}


def _get_program(ncol: int) -> bacc.Bacc:
    if ncol not in _PROGRAM_CACHE:
        _PROGRAM_CACHE[ncol] = _build_program(ncol)
    return _PROGRAM_CACHE[ncol]


def _choose_order(sizes: np.ndarray, nsamples: int = 40000) -> np.ndarray:
    """Pick a class ordering minimizing the max per-core column span."""
    ncls = len(sizes)
    rng = np.random.default_rng(0)
    perms = np.empty((nsamples + 2, ncls), dtype=np.int64)
    perms[0] = np.arange(ncls)
    perms[1] = np.argsort(sizes)[::-1]
    for i in range(nsamples):
        perms[i + 2] = rng.permutation(ncls)
    s = sizes[perms]                                   # [N, ncls]
    pref = np.concatenate(
        [np.zeros((len(perms), 1), np.int64), np.cumsum(s, axis=1)], axis=1)
    a = pref[:, :-1][:, None, :]                       # [N,1,ncls] start
    b = pref[:, 1:][:, None, :]                        # [N,1,ncls] end
    r0 = (np.arange(NCORES) * NR)[None, :, None]
    r1 = r0 + NR
    touch = (a < r1) & (b > r0)
    lo = np.where(touch, a, np.iinfo(np.int64).max).min(axis=2)
    hi = np.where(touch, b, 0).max(axis=2)
    worst = (hi - lo).max(axis=1)
    return perms[int(np.argmin(worst))]


def _prep_host(outputs: np.ndarray, grad: np.ndarray, x: np.ndarray):
    """Class sort, per-core column blocks (own rows first), masks."""
    bf = mybir.dt.np(BF16)
    g = grad.reshape(B, -1).astype(bf)
    xv = x.reshape(B, -1).astype(bf)
    cls = np.argmax(outputs, axis=1)
    ncls = outputs.shape[1]
    sizes = np.bincount(cls, minlength=ncls)
    order = _choose_order(sizes)

    perm = np.concatenate([np.nonzero(cls == c)[0] for c in order])
    pcls = cls[perm]
    pref = np.concatenate([[0], np.cumsum(sizes[order])])

    core_cols = []
    for k in range(NCORES):
        r0, r1 = k * NR, (k + 1) * NR
        # classes sorted -> touched spans form one contiguous range
        ci0 = int(np.searchsorted(pref, r0, side="right")) - 1
        ci1 = int(np.searchsorted(pref, r1 - 1, side="right")) - 1
        lo, hi = int(pref[ci0]), int(pref[ci1 + 1])
        cols = np.concatenate([
            np.arange(r0, r1),          # own rows first
            np.arange(lo, r0),
            np.arange(r1, hi),
        ])
        core_cols.append(cols)

    ncol = ((max(len(c) for c in core_cols) + 127) // 128) * 128

    ident = np.eye(128, dtype=bf)
    in_maps = []
    for k in range(NCORES):
        cols = core_cols[k]
        nreal = len(cols)
        colidx = np.concatenate(
            [cols, np.repeat(cols[-1:], ncol - nreal)])
        rows_global = perm[colidx]                     # original row ids
        gk = g[rows_global]
        xk = xv[rows_global]
        rowcls = pcls[np.arange(k * NR, (k + 1) * NR)]
        colcls = np.full(ncol, -1, dtype=np.int64)
        colcls[:nreal] = pcls[cols]
        gi = np.arange(k * NR, (k + 1) * NR)[:, None]
        gj = np.full(ncol, -2, dtype=np.int64)
        gj[:nreal] = cols
        mask = ((rowcls[:, None] == colcls[None, :])
                & (gi != gj[None, :])).astype(np.float32)
        # own-block (cols < NR are this core's own rows, in the same
        # permuted order): count i<j pairs twice, drop i>j (the kernel
        # skips columns < mi*128 of each m-tile; remaining computed
        # lower-triangle entries inside the diagonal 128-blocks are
        # zeroed here)
        own = mask[:, :NR]
        gi_own = np.arange(NR)[:, None]
        gj_own = np.arange(NR)[None, :]
        own *= np.where(gi_own < gj_own, 2.0, 0.0).astype(np.float32)
        mask = mask.astype(bf)
        in_maps.append({
            "gcols": np.ascontiguousarray(gk),
            "xcols": np.ascontiguousarray(xk),
            "mask": np.ascontiguousarray(mask),
            "ident": ident,
        })
    return ncol, in_maps


def kernel(outputs, grad, x, y):
    outputs = np.asarray(outputs)
    grad = np.asarray(grad)
    x = np.asarray(x)
    ncol, in_maps = _prep_host(outputs, grad, x)
    nc = _get_program(ncol)
    res = bass_utils.run_bass_kernel_spmd(
        nc, in_maps, core_ids=list(range(NCORES)))
    total = float(sum(r["out"][0, 0].astype(np.float64)
                      for r in res.results))
    loss = total / 2.0 / float(B)
    return np.float32(loss)


# revision 14
# speedup vs baseline: 1.1065x; 1.1065x over previous
"""Trainium2 Bass kernel for the pairwise-cosine masked ratio loss.

reference semantics:
    g  = min-max-normalized grad rows          [B, D]
    cos_g, cos_x = pairwise cosine Gram matrices
    loss = sum over same-class pairs i<j of (1-cos_g)/(1-cos_x) / B

Key facts used:
  * cosine is invariant to positive per-row affine scale, so min-max
    normalization reduces to u = (g - rowmin(g)) / ||g - rowmin(g)||.
  * the mask (same argmax class, i<j) makes the Gram sum block-diagonal
    after sorting rows by class; ratio matrix is symmetric, so
    loss = (sum over all same-class ordered pairs i != j) / 2 / B.

Sharding: rows sorted by class (class order chosen to minimize the
max per-core class-span), 512 contiguous sorted rows per core.  Each
core receives its 512 rows plus the rest of the class spans they touch
("column block", rotated so the core's own rows come first), and a
host-built mask [512, NCOL] encoding same-class & not-diagonal &
real-column.  Every same-class ordered pair (i,j) is produced by
exactly one core (the owner of row i), so the masked sum over all
cores counts each unordered pair exactly twice.

Device program (SPMD, identical program for all cores; data differs):
  phase 1 (per matrix): per 128-row tile: row min (g only, fused
           negate), Square activation with accum -> sum of squares;
           batched sqrt + one reciprocal; u = (v - min) * inv -> bf16;
           PE-transpose into U^T [128, KT, NCOL].
  phase 2: per 128-row m-tile: Gram blocks for g and x via bf16
           matmuls (K=1024 in 8 k-tiles, PSUM f32),
           sx = min(cos_x - 1, -1e-30)  (clamp protects masked pairs),
           num = (cos_g - 1) * mask     (DVE),
           partial += sum(num / sx)     (fast-approx reciprocal + fused
           multiply-accumulate on DVE) == sum mask*(1-cos_g)/(1-cos_x).
  finale:  partition-reduce partials via matmul with ones -> [1,1].
Host sums the 8 partial scalars, divides by 2*B.
"""

import numpy as np

import concourse.bass as bass
import concourse.bacc as bacc
import concourse.mybir as mybir
import concourse.tile as tile
from concourse import bass_utils

B = 4096
D = 1024
NCORES = 8
NR = B // NCORES          # 512 own rows per core
KT = D // 128             # k-tiles
MT = NR // 128            # m-tiles per core
F32 = mybir.dt.float32
BF16 = mybir.dt.bfloat16
AF = mybir.ActivationFunctionType
ALU = mybir.AluOpType
AX = mybir.AxisListType


def _build_program(ncol: int) -> bacc.Bacc:
    nc = bacc.Bacc("TRN2", target_bir_lowering=False, debug=False,
                   num_devices=NCORES)
    gcols = nc.dram_tensor("gcols", [ncol, D], BF16, kind="ExternalInput")
    xcols = nc.dram_tensor("xcols", [ncol, D], BF16, kind="ExternalInput")
    maskd = nc.dram_tensor("mask", [NR, ncol], BF16, kind="ExternalInput")
    ident = nc.dram_tensor("ident", [128, 128], BF16, kind="ExternalInput")
    outd = nc.dram_tensor("out", [1, 1], F32, kind="ExternalOutput")

    T = ncol // 128
    # triangle scheme: m-tile mi only computes columns >= mi*128 (the
    # skipped region is the own-block lower triangle; the host mask
    # counts own-block upper pairs with weight 2.0 instead)
    mi_segs = []
    for mi in range(MT):
        segs = []
        cs = mi * 128
        while cs < ncol:
            cw = min(512, ncol - cs)
            segs.append((cs, cw))
            cs += cw
        mi_segs.append(segs)
    npart = sum(len(s) for s in mi_segs)

    with tile.TileContext(nc) as tc:
        with (
            tc.tile_pool(name="cst", bufs=1) as cst,
            tc.tile_pool(name="io", bufs=T + 2) as io,
            tc.tile_pool(name="ut", bufs=1) as utp,
            tc.tile_pool(name="sm", bufs=2) as smp,
            tc.tile_pool(name="wk", bufs=3) as wk,
            tc.tile_pool(name="tp", bufs=2, space="PSUM") as psp,
            tc.tile_pool(name="gr", bufs=2, space="PSUM") as psg,
            tc.tile_pool(name="fi", bufs=1, space="PSUM") as psf,
        ):
            identt = cst.tile([128, 128], BF16, name="identt")
            nc.sync.dma_start(identt[:], ident[:])
            parts = cst.tile([128, npart], F32, name="parts")
            utg = utp.tile([128, KT, ncol], BF16, name="utg")
            utx = utp.tile([128, KT, ncol], BF16, name="utx")

            # ---- phase 1: normalize + transpose (per matrix) ----
            # sqrt/reciprocal are batched per *group* of tiles (two
            # groups per matrix) so downstream applies/transposes can
            # start before the whole matrix is loaded
            for src, ut, submin in ((gcols, utg, True), (xcols, utx, False)):
                nm = smp.tile([128, T], F32, tag="nm", name="nm")
                ssq = smp.tile([128, T], F32, tag="ssq", name="ssq")
                inv = smp.tile([128, T], F32, tag="inv", name="inv")
                groups = [list(range(0, (T + 1) // 2)),
                          list(range((T + 1) // 2, T))]
                for grp in groups:
                    raws = {}
                    for t in grp:
                        raw = io.tile([128, D], BF16, tag="raw", name="raw")
                        nc.sync.dma_start(raw[:],
                                          src[t * 128:(t + 1) * 128, :])
                        raws[t] = raw
                        sq = wk.tile([128, D], BF16, tag="sq", name="sq")
                        if submin:
                            nc.vector.tensor_reduce(nm[:, t:t + 1], raw[:],
                                                    axis=AX.X, op=ALU.min,
                                                    negate=True)
                            nc.scalar.activation(sq[:], raw[:], AF.Square,
                                                 bias=nm[:, t:t + 1],
                                                 scale=1.0,
                                                 accum_out=ssq[:, t:t + 1])
                        else:
                            nc.scalar.activation(sq[:], raw[:], AF.Square,
                                                 bias=0.0, scale=1.0,
                                                 accum_out=ssq[:, t:t + 1])
                    g0, gn = grp[0], len(grp)
                    nrm = smp.tile([128, T], F32, tag="nrm", name="nrm")
                    nc.scalar.sqrt(nrm[:, g0:g0 + gn], ssq[:, g0:g0 + gn])
                    nc.vector.reciprocal(inv[:, g0:g0 + gn],
                                         nrm[:, g0:g0 + gn])
                    for t in grp:
                        u = wk.tile([128, D], BF16, tag="u", name="u")
                        if submin:
                            nc.vector.tensor_scalar(u[:], raws[t][:],
                                                    nm[:, t:t + 1],
                                                    inv[:, t:t + 1],
                                                    op0=ALU.add,
                                                    op1=ALU.mult)
                        else:
                            nc.vector.tensor_scalar_mul(u[:], raws[t][:],
                                                        inv[:, t:t + 1])
                        ps = psp.tile([128, D], BF16, tag="tp", name="ps")
                        for kk in range(KT):
                            nc.tensor.transpose(
                                ps[:, kk * 128:(kk + 1) * 128],
                                u[:, kk * 128:(kk + 1) * 128],
                                identt[:])
                        cp = nc.scalar.copy if t % 2 == 0 else \
                            nc.vector.tensor_copy
                        cp(
                            ut[:, :, t * 128:(t + 1) * 128],
                            ps[:].rearrange("p (k c) -> p k c", k=KT),
                        )

            # ---- phase 2: Gram blocks + masked ratio ----
            pidx = 0
            for mi in range(MT):
                maskt = wk.tile([128, ncol], BF16, tag="maskt", name="maskt")
                nc.sync.dma_start(maskt[:],
                                  maskd[mi * 128:(mi + 1) * 128, :])
                for cs, cw in mi_segs[mi]:
                    pg = psg.tile([128, 512], F32, tag="pg", name="pg")
                    px = psg.tile([128, 512], F32, tag="px", name="px")
                    for kk in range(KT):
                        nc.tensor.matmul(
                            pg[:, :cw],
                            utg[:, kk, mi * 128:(mi + 1) * 128],
                            utg[:, kk, cs:cs + cw],
                            start=(kk == 0), stop=(kk == KT - 1))
                    for kk in range(KT):
                        nc.tensor.matmul(
                            px[:, :cw],
                            utx[:, kk, mi * 128:(mi + 1) * 128],
                            utx[:, kk, cs:cs + cw],
                            start=(kk == 0), stop=(kk == KT - 1))
                    sx = wk.tile([128, 512], F32, tag="sx", name="sx")
                    nc.vector.tensor_scalar(sx[:, :cw], px[:, :cw], 1.0,
                                            -1e-30, op0=ALU.subtract,
                                            op1=ALU.min)
                    rx = wk.tile([128, 512], F32, tag="rx", name="rx")
                    nc.vector.reciprocal_approx_fast(rx[:, :cw], sx[:, :cw])
                    num = wk.tile([128, 512], F32, tag="num", name="num")
                    nc.vector.scalar_tensor_tensor(
                        num[:, :cw], pg[:, :cw], 1.0, maskt[:, cs:cs + cw],
                        op0=ALU.subtract, op1=ALU.mult)
                    junk = wk.tile([128, 512], F32, tag="junk", name="junk")
                    # (num * 1) * rx, accum_out = sum -> partial
                    nc.vector.scalar_tensor_tensor(
                        junk[:, :cw], num[:, :cw], 1.0, rx[:, :cw],
                        op0=ALU.mult, op1=ALU.mult,
                        accum_out=parts[:, pidx:pidx + 1])
                    pidx += 1

            # ---- finale: reduce partials to one scalar ----
            total = smp.tile([128, 1], F32, tag="total", name="total")
            nc.vector.reduce_sum(total[:], parts[:], axis=AX.X)
            ones = cst.tile([128, 1], F32, name="ones")
            nc.vector.memset(ones[:], 1.0)
            fin = psf.tile([1, 1], F32, name="fin")
            nc.tensor.matmul(fin[:], total[:], ones[:])
            outs = smp.tile([1, 1], F32, tag="outs", name="outs")
            nc.scalar.copy(outs[:], fin[:])
            nc.sync.dma_start(outd[:], outs[:])

    nc.compile()
    return nc


_PROGRAM_CACHE: dict = {}


def _get_program(ncol: int) -> bacc.Bacc:
    if ncol not in _PROGRAM_CACHE:
        _PROGRAM_CACHE[ncol] = _build_program(ncol)
    return _PROGRAM_CACHE[ncol]


def _choose_order(sizes: np.ndarray, nsamples: int = 40000) -> np.ndarray:
    """Pick a class ordering minimizing the max per-core column span."""
    ncls = len(sizes)
    rng = np.random.default_rng(0)
    perms = np.empty((nsamples + 2, ncls), dtype=np.int64)
    perms[0] = np.arange(ncls)
    perms[1] = np.argsort(sizes)[::-1]
    for i in range(nsamples):
        perms[i + 2] = rng.permutation(ncls)
    s = sizes[perms]                                   # [N, ncls]
    pref = np.concatenate(
        [np.zeros((len(perms), 1), np.int64), np.cumsum(s, axis=1)], axis=1)
    a = pref[:, :-1][:, None, :]                       # [N,1,ncls] start
    b = pref[:, 1:][:, None, :]                        # [N,1,ncls] end
    r0 = (np.arange(NCORES) * NR)[None, :, None]
    r1 = r0 + NR
    touch = (a < r1) & (b > r0)
    lo = np.where(touch, a, np.iinfo(np.int64).max).min(axis=2)
    hi = np.where(touch, b, 0).max(axis=2)
    worst = (hi - lo).max(axis=1)
    return perms[int(np.argmin(worst))]


def _prep_host(outputs: np.ndarray, grad: np.ndarray, x: np.ndarray):
    """Class sort, per-core column blocks (own rows first), masks."""
    bf = mybir.dt.np(BF16)
    g = grad.reshape(B, -1).astype(bf)
    xv = x.reshape(B, -1).astype(bf)
    cls = np.argmax(outputs, axis=1)
    ncls = outputs.shape[1]
    sizes = np.bincount(cls, minlength=ncls)
    order = _choose_order(sizes)

    perm = np.concatenate([np.nonzero(cls == c)[0] for c in order])
    pcls = cls[perm]
    pref = np.concatenate([[0], np.cumsum(sizes[order])])

    core_cols = []
    for k in range(NCORES):
        r0, r1 = k * NR, (k + 1) * NR
        # classes sorted -> touched spans form one contiguous range
        ci0 = int(np.searchsorted(pref, r0, side="right")) - 1
        ci1 = int(np.searchsorted(pref, r1 - 1, side="right")) - 1
        lo, hi = int(pref[ci0]), int(pref[ci1 + 1])
        cols = np.concatenate([
            np.arange(r0, r1),          # own rows first
            np.arange(lo, r0),
            np.arange(r1, hi),
        ])
        core_cols.append(cols)

    ncol = ((max(len(c) for c in core_cols) + 127) // 128) * 128

    ident = np.eye(128, dtype=bf)
    in_maps = []
    for k in range(NCORES):
        cols = core_cols[k]
        nreal = len(cols)
        colidx = np.concatenate(
            [cols, np.repeat(cols[-1:], ncol - nreal)])
        rows_global = perm[colidx]                     # original row ids
        gk = g[rows_global]
        xk = xv[rows_global]
        rowcls = pcls[np.arange(k * NR, (k + 1) * NR)]
        colcls = np.full(ncol, -1, dtype=np.int64)
        colcls[:nreal] = pcls[cols]
        gi = np.arange(k * NR, (k + 1) * NR)[:, None]
        gj = np.full(ncol, -2, dtype=np.int64)
        gj[:nreal] = cols
        mask = ((rowcls[:, None] == colcls[None, :])
                & (gi != gj[None, :])).astype(np.float32)
        # own-block (cols < NR are this core's own rows, in the same
        # permuted order): count i<j pairs twice, drop i>j (the kernel
        # skips columns < mi*128 of each m-tile; remaining computed
        # lower-triangle entries inside the diagonal 128-blocks are
        # zeroed here)
        own = mask[:, :NR]
        gi_own = np.arange(NR)[:, None]
        gj_own = np.arange(NR)[None, :]
        own *= np.where(gi_own < gj_own, 2.0, 0.0).astype(np.float32)
        mask = mask.astype(bf)
        in_maps.append({
            "gcols": np.ascontiguousarray(gk),
            "xcols": np.ascontiguousarray(xk),
            "mask": np.ascontiguousarray(mask),
            "ident": ident,
        })
    return ncol, in_maps


def kernel(outputs, grad, x, y):
    outputs = np.asarray(outputs)
    grad = np.asarray(grad)
    x = np.asarray(x)
    ncol, in_maps = _prep_host(outputs, grad, x)
    nc = _get_program(ncol)
    res = bass_utils.run_bass_kernel_spmd(
        nc, in_maps, core_ids=list(range(NCORES)))
    total = float(sum(r["out"][0, 0].astype(np.float64)
                      for r in res.results))
    loss = total / 2.0 / float(B)
    return np.float32(loss)
